# revision 1
# baseline (speedup 1.0000x reference)
"""Trainium2 Bass kernel for nn_BayerUpsample4x4.

The reference op: x [4,1,1024,1024] -> 16-channel polyphase 4x bilinear
(tent-filter) upsample, output [4,16,1024,1024].  Each output channel
k=(r,c) is x subsampled at rows≡r, cols≡c (mod 4), zero-upsampled x4 and
convolved with the separable 7x7 tent kernel == bilinear interpolation
with zero padding at image borders.

Kernel plan (per core; 8 cores = 4 batches x 2 row-halves):
  - vertical interpolation on TensorE: fp32 matmul with banded interp
    matrices V (built host-side from `weight`), K=68 subsampled rows
  - PSUM evacuation fused with prescaling on ScalarE: P25/P50/P75
    = 0.25/0.5/0.75 * (vertical result), with 4 zero-pad cols both sides
  - horizontal interpolation as plain adds (measured-optimal split
    between VectorE and GpSimd):  e1 = P75_lo + P25_hi,
    e2 = P50_lo + P50_hi,  e3 = P25_lo + P75_hi
  - e0 columns = 2 * P50 (exact in fp32) on ScalarE
  - final stores are dense 512KB DMAs
"""

import sys
for _p in ("/opt/trn_rl_repo", "/opt/pypackages"):
    if _p not in sys.path:
        sys.path.append(_p)

from contextlib import ExitStack

import numpy as np

import concourse.bass as bass
import concourse.tile as tile
from concourse import bacc, mybir
from concourse.bass_utils import run_bass_kernel_spmd

F32 = mybir.dt.float32
AF = mybir.ActivationFunctionType
OP = mybir.AluOpType

N_CORES = 8
H, W = 1024, 1024
HALF = 512               # output rows per core
SLAB = 528               # padded input slab rows per core
KDIM = 68                # matmul contraction size (subsampled rows + halo)

# (row, col) offset within each 4x4 block for channel k (matches reference)
OFFSETS = [(0, 0), (0, 2), (2, 0), (2, 2),
           (0, 1), (0, 3), (2, 1), (2, 3),
           (1, 0), (1, 2), (3, 0), (3, 2),
           (1, 1), (1, 3), (3, 1), (3, 3)]
K_OF = {rc: k for k, rc in enumerate(OFFSETS)}

# calibrated per-op ns on HW, in-context (FD=256 strided fp32)
_COST_DVE_TT = 550.0
_COST_GPS_TT = 2000.0
_COST_ACT_E0 = 620.0
_COST_ACT_PRE = 550.0


def _emit(tc, xs, vm, out, kh, *, store=True, use_gps=False,
          bufs=(4, 3, 10), qs=(0, 1)):
    """Trace the per-core program.

    xs:  [528, 1024] f32 zero-padded input slab (rows h0-4 .. h0+523)
    vm:  [8, 68, 128] f32 vertical interp matrices, index r*2+b, [p, m]
    out: [16, 512, 1024] f32
    kh:  length-7 horizontal filter profile (numpy)
    """
    nc = tc.nc
    b_e = {e: float(kh[7 - e]) for e in (1, 2, 3)}   # 0.25 / 0.5 / 0.75

    load = {"dve": 0.0, "gps": 0.0, "act": 0.0}   # greedy engine balance
    if not use_gps:
        load["gps"] = 1e12
    setno = 0   # tile-set counter (for one-time pad init per pool slot)

    with ExitStack() as ctx:
        vpool = ctx.enter_context(tc.tile_pool(name="vmp", bufs=1))
        xpool = ctx.enter_context(tc.tile_pool(name="xp", bufs=5))
        pspool = ctx.enter_context(tc.tile_pool(name="psp", bufs=bufs[0],
                                                space="PSUM"))
        vtpool = ctx.enter_context(tc.tile_pool(name="vtp", bufs=bufs[1]))
        opool = ctx.enter_context(tc.tile_pool(name="op", bufs=bufs[2]))

        # ---- load all 8 V matrices into one [68, 8*128] tile ----
        vmt = vpool.tile([KDIM, 8 * 128], F32, tag="vmt")
        nc.sync.dma_start(vmt[:], vm.rearrange("i p m -> p i m"))

        xs_rows = xs.rearrange("(t s) w -> s t w", s=4)   # [4, 132, 1024]

        for q in qs:
            for r in range(4):
                xt = xpool.tile([KDIM, W], F32, tag="xt")
                nc.sync.dma_start(xt[:], xs_rows[r][64 * q: 64 * q + KDIM, :])

                for b in range(2):
                    lhsT = vmt[:, (r * 2 + b) * 128: (r * 2 + b + 1) * 128]

                    # prescaled vertical results; 4 zero pad cols both sides
                    p25 = vtpool.tile([128, W + 8], F32, tag="p25")
                    p50 = vtpool.tile([128, W + 8], F32, tag="p50")
                    p75 = vtpool.tile([128, W + 8], F32, tag="p75")
                    for t in (p25, p50, p75):
                        pad = t.rearrange("p (g u) -> p g u", u=4)
                        nc.vector.memset(pad[:, 0:258:257, :], 0.0)
                    setno += 1

                    pss = []
                    for ch in range(2):
                        ps = pspool.tile([128, 512], F32, tag="ps")
                        nc.tensor.matmul(
                            ps[:], lhsT=lhsT,
                            rhs=xt[:, 512 * ch: 512 * ch + 512],
                            start=True, stop=True,
                        )
                        pss.append(ps)
                    # p50 first across both chunks: e0/e2 consumers depend
                    # only on it and can start after two ACT ops
                    for scale, arr in ((b_e[2], p50), (b_e[1], p25),
                                       (b_e[3], p75)):
                        for ch in range(2):
                            dl = slice(4 + 512 * ch, 4 + 512 * ch + 512)
                            nc.scalar.activation(arr[:, dl], pss[ch][:],
                                                 AF.Copy, scale=scale)
                            load["act"] += _COST_ACT_PRE

                    # grouped [128, 258, 4] views for phase-strided access
                    pv = {1: p25.rearrange("p (u s) -> p u s", s=4),
                          2: p50.rearrange("p (u s) -> p u s", s=4),
                          3: p75.rearrange("p (u s) -> p u s", s=4)}

                    for c in range(4):
                        k = K_OF[(r, c)]
                        oc = opool.tile([128, W], F32, tag="oc")
                        ov = oc.rearrange("p (u s) -> p u s", s=4)
                        # e = 0: out phase c = Vt = 2*P50 (P50+P50 as TT keeps
                        # DVE/GpSimd in 1-port mode -> no shared-port lock)
                        u0, s0 = divmod(4 + c, 4)
                        src = pv[2][:, u0:u0 + 256, s0]
                        picks = {"act": load["act"] + _COST_ACT_E0,
                                 "dve": load["dve"] + _COST_DVE_TT,
                                 "gps": load["gps"] + _COST_GPS_TT}
                        eng = min(picks, key=picks.get)
                        load[eng] = picks[eng]
                        if eng == "act":
                            nc.scalar.activation(ov[:, :, c], src,
                                                 AF.Copy, scale=2.0)
                        elif eng == "dve":
                            nc.vector.tensor_tensor(ov[:, :, c], src, src,
                                                    OP.add)
                        else:
                            nc.gpsimd.tensor_tensor(ov[:, :, c], src, src,
                                                    OP.add)
                        for e in (1, 2, 3):
                            j0 = (c + e) % 4
                            st = 4 + j0 - e          # lo col start (1..6)
                            u0, s0 = divmod(st, 4)
                            u1, s1 = divmod(st + 4, 4)
                            lo = pv[4 - e][:, u0:u0 + 256, s0]
                            hi = pv[e][:, u1:u1 + 256, s1]
                            if load["dve"] + _COST_DVE_TT <= \
                                    load["gps"] + _COST_GPS_TT:
                                load["dve"] += _COST_DVE_TT
                                eng2 = nc.vector
                            else:
                                load["gps"] += _COST_GPS_TT
                                eng2 = nc.gpsimd
                            eng2.tensor_tensor(ov[:, :, j0], lo, hi, OP.add)
                        if store:
                            row0 = 256 * q + 128 * b
                            nc.sync.dma_start(out[k, row0:row0 + 128, :],
                                              oc[:])


_CACHE = {}


def _build_module(kh):
    key = tuple(np.asarray(kh, np.float64).tolist())
    if key in _CACHE:
        return _CACHE[key]
    nc = bacc.Bacc("TRN2", target_bir_lowering=False, debug=False)
    xs = nc.dram_tensor("xs", [SLAB, W], F32, kind="ExternalInput").ap()
    vm = nc.dram_tensor("vm", [8, KDIM, 128], F32, kind="ExternalInput").ap()
    out = nc.dram_tensor("out", [16, HALF, W], F32, kind="ExternalOutput").ap()
    with tile.TileContext(nc) as tc:
        _emit(tc, xs, vm, out, kh)
    nc.compile()
    _CACHE[key] = nc
    return nc


def _vmats(kv):
    V = np.zeros((8, KDIM, 128), np.float32)
    for r in range(4):
        for b in range(2):
            for m in range(128):
                d = (m - r) % 4
                p_lo = 32 * b + (m - r - d) // 4 + 1
                V[r * 2 + b, p_lo, m] += kv[3 - d]
                if d > 0:
                    V[r * 2 + b, p_lo + 1, m] += kv[7 - d]
    return V


def _slabs(x):
    s = np.zeros((N_CORES, SLAB, W), np.float32)
    for core in range(N_CORES):
        n, half = divmod(core, 2)
        g0 = 512 * half - 4
        s0, s1 = max(0, g0), min(H, g0 + SLAB)
        s[core, s0 - g0: s1 - g0] = x[n, 0, s0:s1]
    return s


def kernel(x, weight):
    x = np.asarray(x, np.float32)
    weight = np.asarray(weight, np.float32)
    assert x.shape == (4, 1, H, W), x.shape
    k2 = weight[0, 0]
    kv = k2[:, 3].astype(np.float64)   # vertical profile (k1)
    kh = k2[3, :].astype(np.float64)   # horizontal profile (k1)

    nc = _build_module(kh)
    V = _vmats(kv)
    slabs = _slabs(x)
    in_maps = [{"xs": slabs[c], "vm": V} for c in range(N_CORES)]
    res = run_bass_kernel_spmd(nc, in_maps, list(range(N_CORES)))

    full = np.empty((4, 16, H, W), np.float32)
    for core in range(N_CORES):
        n, half = divmod(core, 2)
        full[n, :, 512 * half: 512 * half + 512, :] = res.results[core]["out"]
    return full



# revision 2
# speedup vs baseline: 1.5208x; 1.5208x over previous
"""Trainium2 Bass kernel for nn_BayerUpsample4x4 — two-pass all-matmul design.

The reference op: x [4,1,1024,1024] -> 16-channel polyphase 4x bilinear
(tent-filter) upsample, output [4,16,1024,1024].  Channel k=(r,c) is x
subsampled at rows=r, cols=c (mod 4), zero-upsampled x4 and convolved
with the separable 7x7 tent kernel (zero padding at borders).

Kernel plan (8 cores = 4 batches x 2 row-halves; 512 out rows/core):
  The separable interp is two matmul contractions.  The host pre-builds a
  TRANSPOSED, phase-separated copy of the input so that BOTH passes
  contract over the partition dim (no on-chip transposes, no strided
  vector ops):

  pass 1 (horizontal):  Z_cr[v, q] = sum_t kh[q-(4t+c)] * x[4v+r, 4t+c]
      matmul: lhsT = xt tile [K=66 lattice cols, M = lattice rows],
              rhs  = banded const H_c [66, 256 out cols]
  pass 2 (vertical):    out[p, q] = sum_v kv[p-(4v+r)] * Z_cr[v, q]
      matmul: lhsT = banded const V_rb [K, M=128 out rows],
              rhs  = Z_cr [K, 512 out cols]

  All matmul operands bf16 (1 cyc/row), PSUM fp32.  PSUM evacuation on
  ScalarE+VectorE (dense, FD>=512).  One dense 2 MB DMA store per
  channel.  Everything overlaps behind the ~90us/core HBM store floor.

  Lattice rows per (r): ~130 -> an A-chunk (128 rows) serving out-blocks
  b=0..2 via band-embedded V (K=128), and a B-chunk (34 rows) serving
  b=3 (K=34).  Rows duplicated between chunks keep every pass-2 rhs
  window partition-contiguous from partition 0.
"""

import sys
for _p in ("/opt/trn_rl_repo", "/opt/pypackages"):
    if _p not in sys.path:
        sys.path.append(_p)

from contextlib import ExitStack

import numpy as np
import ml_dtypes

import concourse.bass as bass
import concourse.tile as tile
from concourse import bacc, mybir
from concourse.bass_utils import run_bass_kernel_spmd

F32 = mybir.dt.float32
BF16 = mybir.dt.bfloat16
AF = mybir.ActivationFunctionType

N_CORES = 8
H, W = 1024, 1024
HALF = 512                # output rows per core
KT = 66                   # pass-1 contraction (lattice-col window)
NA = 128                  # A-chunk lattice rows
NB = 34                   # B-chunk lattice rows
CPRC = 4 * NA + 4 * NB    # xt cols per (r,c): 4 j-windows each of A and B
XT_W = 16 * CPRC          # 10368

# (row, col) offset within each 4x4 block for channel k (matches reference)
OFFSETS = [(0, 0), (0, 2), (2, 0), (2, 2),
           (0, 1), (0, 3), (2, 1), (2, 3),
           (1, 0), (1, 2), (3, 0), (3, 2),
           (1, 1), (1, 3), (3, 1), (3, 3)]
K_OF = {rc: k for k, rc in enumerate(OFFSETS)}


def _ceil_div(a, b):
    return -((-a) // b)


def _v_lo(P0, r):
    return _ceil_div(P0 - 3 - r, 4)


def _emit(tc, xs, hm, vm, out):
    """Trace the per-core program.

    xs:  [66, 10368] bf16 packed pass-1 lhsT tiles (host-built)
    hm:  [66, 1024]  bf16 horizontal interp matrices (4x [66,256])
    vm:  [128, 2048] bf16 vertical interp matrices (16x [128,128], (r,b))
    out: [16, 512, 1024] f32
    """
    nc = tc.nc

    with ExitStack() as ctx:
        cpool = ctx.enter_context(tc.tile_pool(name="const", bufs=1))
        pApool = ctx.enter_context(tc.tile_pool(name="psA", bufs=1,
                                                space="PSUM"))
        pBpool = ctx.enter_context(tc.tile_pool(name="psB", bufs=1,
                                                space="PSUM"))
        pOpool = ctx.enter_context(tc.tile_pool(name="psO", bufs=3,
                                                space="PSUM"))
        zpool = ctx.enter_context(tc.tile_pool(name="z", bufs=2))
        opool = ctx.enter_context(tc.tile_pool(name="o", bufs=2))

        xt = cpool.tile([KT, XT_W], BF16, tag="xt")
        nc.sync.dma_start(xt[:], xs)
        hmt = cpool.tile([KT, 1024], BF16, tag="hm")
        nc.sync.dma_start(hmt[:], hm)
        vmt = cpool.tile([128, 16 * 128], BF16, tag="vm")
        nc.sync.dma_start(vmt[:], vm)

        evac_flip = 0
        for r in range(4):
            for c in range(4):
                k = K_OF[(r, c)]
                base = (r * 4 + c) * CPRC
                Hc = hmt[:, 256 * c: 256 * c + 256]

                psA = pApool.tile([NA, 1024], F32, tag="psA")
                psB = pBpool.tile([NB, 1024], F32, tag="psB")
                for j in range(4):
                    nc.tensor.matmul(
                        psA[:, 256 * j: 256 * j + 256],
                        lhsT=xt[:, base + NA * j: base + NA * j + NA],
                        rhs=Hc, start=True, stop=True)
                    bb = base + 4 * NA
                    nc.tensor.matmul(
                        psB[:, 256 * j: 256 * j + 256],
                        lhsT=xt[:, bb + NB * j: bb + NB * j + NB],
                        rhs=Hc, start=True, stop=True)

                zA = zpool.tile([NA, 1024], BF16, tag="zA")
                zB = zpool.tile([NB, 1024], BF16, tag="zB")
                nc.scalar.activation(zA[:, 0:512], psA[:, 0:512], AF.Copy)
                nc.vector.tensor_copy(zA[:, 512:1024], psA[:, 512:1024])
                nc.scalar.activation(zB[:], psB[:], AF.Copy)

                ot = opool.tile([128, 4096], F32, tag="ot")
                for b in range(4):
                    vblk = vmt[:, (r * 4 + b) * 128: (r * 4 + b) * 128 + 128]
                    if b < 3:
                        lhsT, rsrc = vblk, zA
                    else:
                        lhsT, rsrc = vblk[0:NB, :], zB
                    for nchk in range(2):
                        psO = pOpool.tile([128, 512], F32, tag="psO")
                        nc.tensor.matmul(
                            psO[:], lhsT=lhsT,
                            rhs=rsrc[:, 512 * nchk: 512 * nchk + 512],
                            start=True, stop=True)
                        dst = ot[:, 1024 * b + 512 * nchk:
                                 1024 * b + 512 * nchk + 512]
                        if evac_flip % 2 == 0:
                            nc.scalar.activation(dst, psO[:], AF.Copy)
                        else:
                            nc.vector.tensor_copy(dst, psO[:])
                        evac_flip += 1

                nc.sync.dma_start(
                    out[k].rearrange("(b p) q -> p b q", b=4),
                    ot[:].rearrange("p (b q) -> p b q", q=1024))


_CACHE = {}


def _build_module():
    if "m" in _CACHE:
        return _CACHE["m"]
    nc = bacc.Bacc("TRN2", target_bir_lowering=False, debug=False)
    xs = nc.dram_tensor("xs", [KT, XT_W], BF16, kind="ExternalInput").ap()
    hm = nc.dram_tensor("hm", [KT, 1024], BF16, kind="ExternalInput").ap()
    vm = nc.dram_tensor("vm", [128, 16 * 128], BF16,
                        kind="ExternalInput").ap()
    out = nc.dram_tensor("out", [16, HALF, W], F32,
                         kind="ExternalOutput").ap()
    with tile.TileContext(nc) as tc:
        _emit(tc, xs, hm, vm, out)
    nc.compile()
    _CACHE["m"] = nc
    return nc


def _hmat(kh):
    """[66, 1024] f32: 4 horizontal interp blocks H_c [66, 256].

    H_c[t', q'] = kh[7 + q' - 4t' - c] where in [0,7); out col q = 256j+q'
    reads lattice col 4(64j-1+t') + c (zero-padded xt rows handle borders).
    """
    hm = np.zeros((KT, 1024), np.float32)
    tp = np.arange(KT)
    qp = np.arange(256)
    for c in range(4):
        e = 7 + qp[None, :] - 4 * tp[:, None] - c
        m = (e >= 0) & (e <= 6)
        hm[:, 256 * c: 256 * c + 256][m] = kh[e[m]]
    return hm


def _vmat(kv, half):
    """[128, 2048] f32: 16 vertical blocks V_(r,b) [128, 128].

    b<3: band-embedded, V[s, m] = kv[3 + (P0+128b+m) - (4(v_lo+s)+r)].
    b=3: rows 0..33 for the B-chunk (v = v_lo+96+s), rest zero.
    """
    P0 = 512 * half
    vm = np.zeros((128, 16 * 128), np.float32)
    mm = np.arange(128)
    for r in range(4):
        vlo = _v_lo(P0, r)
        for b in range(4):
            s = np.arange(128 if b < 3 else NB)
            vbase = vlo if b < 3 else vlo + 96
            d = 3 + (P0 + 128 * b + mm[None, :]) \
                - (4 * (vbase + s[:, None]) + r)
            msk = (d >= 0) & (d <= 6)
            blk = np.zeros((128, 128), np.float32)
            sub = np.zeros(d.shape, np.float32)
            sub[msk] = kv[d[msk]]
            blk[: d.shape[0]] = sub
            vm[:, (r * 4 + b) * 128: (r * 4 + b) * 128 + 128] = blk
    return vm


def _xt_core(xp, half):
    """[66, 10368] f32 packed pass-1 lhsT tiles for one (image, half).

    xp: [1040, 1032] zero-padded image, xp[4+i, 4+j] = x[i, j].
    Tile (r,c,chunk,j)[t', m] = x[4(vbase+m)+r, 256j - 4 + 4t' + c].
    """
    P0 = 512 * half
    xt = np.zeros((KT, XT_W), np.float32)
    tp4 = 4 * np.arange(KT)
    for r in range(4):
        vlo = _v_lo(P0, r)
        for c in range(4):
            base = (r * 4 + c) * CPRC
            for j in range(4):
                cols = 256 * j + tp4 + c          # +4 pad -4 offset
                rowsA = 4 * (vlo + np.arange(NA)) + r + 4
                xt[:, base + NA * j: base + NA * j + NA] = \
                    xp[np.ix_(rowsA, cols)].T
                rowsB = 4 * (vlo + 96 + np.arange(NB)) + r + 4
                bb = base + 4 * NA
                xt[:, bb + NB * j: bb + NB * j + NB] = \
                    xp[np.ix_(rowsB, cols)].T
    return xt


def _host_inputs(x, weight):
    x = np.asarray(x, np.float32)
    weight = np.asarray(weight, np.float32)
    k2 = weight[0, 0]
    kv = k2[:, 3].astype(np.float32)
    kh = k2[3, :].astype(np.float32)

    bf = ml_dtypes.bfloat16
    hm = _hmat(kh).astype(bf)
    vms = [_vmat(kv, h).astype(bf) for h in range(2)]

    in_maps = []
    for core in range(N_CORES):
        n, half = divmod(core, 2)
        xp = np.zeros((H + 16, W + 8), np.float32)
        xp[4:4 + H, 4:4 + W] = x[n, 0]
        in_maps.append({"xs": _xt_core(xp, half).astype(bf),
                        "hm": hm, "vm": vms[half]})
    return in_maps


def kernel(x, weight):
    assert np.asarray(x).shape == (4, 1, H, W)
    nc = _build_module()
    in_maps = _host_inputs(x, weight)
    res = run_bass_kernel_spmd(nc, in_maps, list(range(N_CORES)))

    full = np.empty((4, 16, H, W), np.float32)
    for core in range(N_CORES):
        n, half = divmod(core, 2)
        full[n, :, 512 * half: 512 * half + 512, :] = \
            res.results[core]["out"]
    return full
